# revision 8
# baseline (speedup 1.0000x reference)
"""GroupWiseLinear Trainium2 kernel.

out[b, c] = dot(W[0, c, :], x[b, group_of[c], :]) + bias[0, c], then a final
class-permutation gather, for two independent branches (co / cl).

Sharding: the 128 ragged group-segments (64 per branch) are chopped into
pieces of <= T classes and LPT-assigned across all 8 cores (cores freely mix
branches; the host unshard step composes the final permutation and adds the
bias, so class placement is arbitrary).  Every core runs the SAME program over
S slots whose widths come from a shared profile = elementwise max of each
core's sorted piece widths (rounded up to GRAN), so the instruction stream is
SPMD-uniform while W columns are only padded by the profile slack.

Device layout per core (bf16):
  - xw: [128, NXW]   chunked slabs, each = per-slot x^T stationaries
        ([128, 64] per (slot, k-chunk)) followed by the chunk's W^T columns.
        One DMA per chunk, all on the SP queue so transfers stay in slot
        order; the final chunk is a single narrow slot so almost no compute
        remains after the last byte lands.
  - o:  [64, totW]   output (batch-major), bf16; bias is added on host.

Per slot: 4 accumulating matmuls (x stationary [128,64], W moving [128,w])
into a PSUM bank region.  PSUM banks are packed first-fit with 512-col
capacity; each (chunk x bank) piece is copied f32->bf16 by the DVE as soon as
its slots finish, and two output DMAs (everything-but-last-piece, then the
last piece) keep the critical tail short.
"""

import ml_dtypes
import numpy as np

import concourse.bacc as bacc
import concourse.tile as tile
from concourse import mybir
from concourse.bass_utils import run_bass_kernel_spmd

B = 64          # batch
H = 512         # hidden
G = 64          # groups per branch
KC = H // 128   # contraction chunks
NCORES = 8
CAP = 1024      # class columns per core (2 * 4096 / 8)
T = 256         # max piece width (chop granularity)
GRAN = 16      # slot width granularity
PSUM_COLS = 512
NCHUNK = 4      # slab DMAs over slots 0..S-2 (last slot is its own chunk)

_cache = {}


def _segments(go):
    """Runs of equal group id -> list of (group, class_start, length)."""
    go = np.asarray(go).astype(np.int64)
    segs = []
    n = len(go)
    i = 0
    while i < n:
        g = int(go[i])
        j = i
        while j < n and go[j] == g:
            j += 1
        segs.append((g, i, j - i))
        i = j
    return segs


def _plan(co_group_of, cl_group_of):
    """Chop segments, LPT-assign pieces to cores, build the shared profile."""
    pieces = []
    for b, go in ((0, co_group_of), (1, cl_group_of)):
        for g, st, L in _segments(go):
            off = 0
            while L > 0:
                w = min(T, L)
                pieces.append((b, g, st + off, w))
                off += w
                L -= w
    pieces.sort(key=lambda p: (-p[3], p[0], p[2]))
    loads = [0] * NCORES
    assign = [[] for _ in range(NCORES)]
    for p in pieces:
        c = min(range(NCORES), key=lambda k: (loads[k] + p[3] > CAP, loads[k], k))
        w = p[3]
        if loads[c] + w > CAP:
            room = CAP - loads[c]
            if room > 0:
                assign[c].append((p[0], p[1], p[2], room))
                loads[c] += room
                p = (p[0], p[1], p[2] + room, w - room)
            c = min(range(NCORES), key=lambda k: (loads[k], k))
        assign[c].append(p)
        loads[c] += p[3]
    for a in assign:
        a.sort(key=lambda p: (-p[3], p[0], p[2]))
    S = max(len(a) for a in assign)
    prof = []
    for i in range(S):
        m = max((a[i][3] if i < len(a) else 0) for a in assign)
        prof.append(int(-(-m // GRAN) * GRAN))
    return assign, prof


def _layout(prof):
    """PSUM bank packing + DMA chunking + column offsets, all profile-only."""
    S = len(prof)
    goff = [0]
    for w in prof:
        goff.append(goff[-1] + w)
    totW = goff[-1]

    # psum banks: sequential first-fit, 512-col capacity: (first_slot, used, base)
    banks = []
    cur_used = 0
    cur_base = 0
    cur_first = 0
    slot_bank = []
    for j, w in enumerate(prof):
        if cur_used + w > PSUM_COLS:
            banks.append((cur_first, cur_used, cur_base))
            cur_first = j
            cur_base = goff[j]
            cur_used = 0
        slot_bank.append(len(banks))
        cur_used += w
    banks.append((cur_first, cur_used, cur_base))

    # DMA chunks: NCHUNK equal-byte slabs over slots 0..S-2, then {S-1} alone
    chunks = []
    if S == 1:
        chunks = [(0, 1)]
    else:
        tot_bytes = sum((64 + prof[j]) * 1024 for j in range(S - 1))
        target = tot_bytes / NCHUNK
        lo = 0
        acc = 0
        for j in range(S - 1):
            acc += (64 + prof[j]) * 1024
            if acc >= target * (len(chunks) + 1) - 1 or j == S - 2:
                chunks.append((lo, j + 1))
                lo = j + 1
        chunks = [c for c in chunks if c[0] < c[1]]
        chunks.append((S - 1, S))
    return goff, totW, slot_bank, banks, chunks


def _program(prof, dt=mybir.dt.bfloat16):
    S = len(prof)
    goff, totW, slot_bank, banks, chunks = _layout(prof)
    nb = len(banks)
    nxw = sum((hi - lo) * KC * 64 + KC * (goff[hi] - goff[lo]) for lo, hi in chunks)

    nc = bacc.Bacc("TRN2", target_bir_lowering=False, debug=False, num_devices=8)
    xw_d = nc.dram_tensor("xw", [128, nxw], dt, kind="ExternalInput")
    o_d = nc.dram_tensor("o", [64, totW], dt, kind="ExternalOutput")

    lastA = goff[chunks[-1][0]]  # columns [0, lastA) go in the first out DMA

    with tile.TileContext(nc) as tc:
        with (
            tc.tile_pool(name="sb", bufs=1) as sb,
            tc.tile_pool(name="ps", bufs=1, space="PSUM") as ps,
        ):
            # input slabs, all on the SP queue in slot order
            xw_tiles = []
            dbase = 0
            for ci, (lo, hi) in enumerate(chunks):
                cols = goff[hi] - goff[lo]
                ccols = (hi - lo) * KC * 64 + KC * cols
                xw = sb.tile([128, ccols], dt, tag=f"xw{ci}", name=f"xw{ci}")
                nc.sync.dma_start(xw[:], xw_d[:, dbase : dbase + ccols])
                xw_tiles.append((xw, lo, hi, (hi - lo) * KC * 64, cols))
                dbase += ccols

            pbanks = [
                ps.tile([64, PSUM_COLS], mybir.dt.float32, tag=f"pb{i}", name=f"pb{i}")
                for i in range(nb)
            ]
            ob = sb.tile([64, totW], dt, tag="ob")

            for ci, (xw, lo, hi, woff, cols) in enumerate(xw_tiles):
                for j in range(lo, hi):
                    w = prof[j]
                    bi = slot_bank[j]
                    bfirst, bused, bbase = banks[bi]
                    po = goff[j] - bbase
                    loc = goff[j] - goff[lo]
                    acc = pbanks[bi]
                    for k in range(KC):
                        nc.tensor.matmul(
                            acc[0:64, po : po + w],
                            xw[:, ((j - lo) * KC + k) * 64 : ((j - lo) * KC + k + 1) * 64],
                            xw[:, woff + k * cols + loc : woff + k * cols + loc + w],
                            start=(k == 0),
                            stop=(k == KC - 1),
                        )
                    # copy each finished (chunk x bank) piece
                    if j == S - 1 or j == hi - 1 or slot_bank[j + 1] != bi:
                        a0 = max(lo, bfirst)
                        nc.vector.tensor_copy(
                            ob[0:64, goff[a0] : goff[j] + w],
                            acc[0:64, goff[a0] - bbase : po + w],
                        )
                        if goff[j] + w == lastA:
                            nc.scalar.dma_start(o_d[:, 0:lastA], ob[0:64, 0:lastA])
            nc.sync.dma_start(o_d[:, lastA:totW], ob[0:64, lastA:totW])

    nc.compile()
    return nc


def _host_prep(x, Ws, pieces, prof):
    """Build xw for one core.  pieces: list of (branch, group, cls0, w)."""
    goff, totW, slot_bank, banks, chunks = _layout(prof)
    nxw = sum((hi - lo) * KC * 64 + KC * (goff[hi] - goff[lo]) for lo, hi in chunks)
    xw = np.zeros((128, nxw), ml_dtypes.bfloat16)
    dbase = 0
    for lo, hi in chunks:
        cols = goff[hi] - goff[lo]
        woff = dbase + (hi - lo) * KC * 64
        for j in range(lo, min(hi, len(pieces))):
            b, g, cls0, wr = pieces[j]
            xs = x[:, b * G + g, :].reshape(B, KC, 128).transpose(2, 1, 0).reshape(128, KC * 64)
            xw[:, dbase + (j - lo) * KC * 64 : dbase + (j - lo + 1) * KC * 64] = xs
            wseg = Ws[b][cls0 : cls0 + wr, :].reshape(wr, KC, 128).transpose(2, 1, 0)
            loc = goff[j] - goff[lo]
            for k in range(KC):
                xw[:, woff + k * cols + loc : woff + k * cols + loc + wr] = wseg[:, k, :]
        dbase += (hi - lo) * KC * 64 + KC * cols
    return {"xw": xw}


def kernel(x, co_W, cl_W, co_b, cl_b, co_group_of, cl_group_of, co_index,
           cl_index, group_len, _return_raw=False):
    x = np.asarray(x, np.float32)
    assign, prof = _plan(co_group_of, cl_group_of)
    goff, totW, slot_bank, banks, chunks = _layout(prof)

    key = ("v4", tuple(prof))
    if key not in _cache:
        _cache.clear()
        _cache[key] = _program(prof)
    nc = _cache[key]

    Ws = (np.asarray(co_W, np.float32)[0], np.asarray(cl_W, np.float32)[0])
    bs = (np.asarray(co_b, np.float32)[0], np.asarray(cl_b, np.float32)[0])
    in_maps = [_host_prep(x, Ws, assign[c], prof) for c in range(NCORES)]

    res = run_bass_kernel_spmd(nc, in_maps, list(range(NCORES)))

    NC_CLS = len(np.asarray(co_group_of))
    fulls = [np.empty((B, NC_CLS), np.float32) for _ in range(2)]
    for c in range(NCORES):
        o = np.asarray(res.results[c]["o"], ml_dtypes.bfloat16).astype(np.float32)
        for j, (b, g, cls0, wr) in enumerate(assign[c]):
            fulls[b][:, cls0 : cls0 + wr] = o[:, goff[j] : goff[j] + wr]
    fulls[0] += bs[0][None, :]
    fulls[1] += bs[1][None, :]
    co_out = fulls[0][:, np.asarray(co_index).astype(np.int64)]
    cl_out = fulls[1][:, np.asarray(cl_index).astype(np.int64)]
    return co_out, cl_out
